# revision 67
# baseline (speedup 1.0000x reference)
"""Multi-head attention (nonstandard softmax normalization) on 8 Trainium2 cores.

Reference computation (B=4, E=1024, S=1024, H=16, HS=64):
  per (b, h):  q = Wq[h] @ Q_h,  k = Wk[h] @ K_h,  v = Wv[h] @ V_h   (feature-first [HS, S])
               pre[s,t] = q[:,s]. k[:,t] / 8
               e = exp(pre);  denom[t] = sum_u e[t,u];  post[s,t] = e[s,t] / denom[t]
               out_h = v @ post.T                                     ([HS, S])
  out = concat_h(out_h);  result[b] = Wo @ out[b]
Sharding: core c -> (b = c//2, head-group hg = c%2 of 8 heads).  Each core
computes its 8 heads end-to-end plus the partial Wo product over its 512
rows; the host sums the partial products per batch (in fp32; the
partials travel as fp16).

On-chip layout notes:
 - Wq is folded into the K projection on the host (pre = q_raw^T
   (Wq^T Wk) k_raw), so only K/V need on-chip projections and QK's moving
   operand is the RAW q rows straight from DRAM.
 - heads are processed in pairs; per-pair blockdiag [128,128] weight tiles
   (N_h = Wq^T Wk, and Wv) let the projections contract over the full 128
   partitions.
 - QK^T is computed transposed (preT[t,s]) so exp(preT) ("E") has t on
   partitions, which is what the AV matmul needs; E chunks use the
   interleaved t mapping t = c*128 + p.
 - the denominator (partition-dim sums of E) is computed with a DoubleRow
   fp8 ones-matmul that reads the HIGH BYTES of fp16 tiles through a
   stride-2 float8e5 bitcast view (an fp16's high byte IS its e5m2
   truncation).  For the three off-tail pairs, the DVE pre-sums chunk
   pairs (tree_a) so the PE only contracts 4 planes (tree_b); the last
   pair uses the direct 8-plane form for the lowest tail latency.  The
   truncation bias (sum_trunc ~ 0.91576 * sum, identical for E values and
   their pairwise sums) is corrected by pre-scaling Wv by CORR on the
   host; the residual error is ~0.13% random.  fp8 anywhere ELSE (AV/Wo
   operands) fails the 2e-2 gate: those are random-sign sums, so fp8
   noise does NOT average down (measured 4-7% end-to-end).
 - the denominator row (free-dim layout, 32 identical psum rows) becomes
   partition-major [128, C] entirely on-chip: one DVE 32x32-block
   StreamTranspose + 4 strided picks; recip(denom) is folded into v^T.
   No DRAM round-trip.
 - QK stationaries are zero-padded to K=128 per head: K=64 matmuls stream
   at HALF rate on trn2 (measured again: 427ns vs 213ns at N=512), while
   a 128-partition stationary whose other half is zeros runs at full
   rate.  tile_position row/col packing does NOT make matmuls concurrent
   on this toolchain (0 overlapping MMs in any trace), so the padded form
   is the optimum.  A matmul's fp32 psum output cannot cross a bank
   (N<=512), so N=1024 merging is off the table too.
 - Wo is emitted as two independent fc-half projections into two DRAM
   partials (out_pa = fc01, out_pb = fc23) that the host sums during
   unshard; fc01 chunks run inside the exp-bound windows (after av(1))
   and the leftovers pad the tail's denominator latency.
 - startup: exp ACT-table preload + ~24 junk matmuls keep the ACT table
   resident and the PE HAM clock-gate at 8/8 through the DMA-bound head
   (~18us: the per-descriptor DMA floor of the critical w/k/q tiles).
 - PSUM pools: pqk 2x[128,1024] (QK->exp, exclusively), psm 2x[128,512]
   (proj/av/wo), dpsp 2x[32,512] (denominators) -- denominators must NOT
   share the QK pool or they stall the exp stream.
 - PSUM hazard note: accumulation chains that interleave at different FREE
   offsets within a bank corrupt results on HW (verified again: chaining
   two s-halves of one stationary NaNs); chains here are emitted
   contiguously (interleaving at different partition offsets is fine).
"""

import os
import sys
import types

import numpy as np

import concourse.bass as bass
import concourse.mybir as mybir
import concourse.tile as tile
from contextlib import ExitStack

B, E, S_FULL, H = 4, 1024, 1024, 16
HS = 64
N_CORES = 8
HEADS_PER_CORE = H // 2          # 8: head-group per core
N_PAIRS_FULL = HEADS_PER_CORE // 2  # 4

# E[e5m2-truncation ratio] for lognormal(0,1)-distributed E values:
# denom_trunc = 0.91576*denom, so recip(denom_trunc) = 1.092/denom and the
# product CORR * recip(denom_trunc) = 1/denom.
CORR = 0.91576

_f32 = mybir.dt.float32
_f16 = mybir.dt.float16
_f8 = mybir.dt.float8e5


def _install_ntff_shim():
    """Register the axon NTFF profile hook if the image's antenv lacks it.

    Lets run_bass_kernel_spmd(trace=True) return exec_time_ns. Harmless if
    already present.
    """
    try:
        import antenv.axon_hooks  # noqa: F401
        return
    except ImportError:
        pass
    try:
        import antenv
        from trn_agent_boot.trn_boot import _ntff_profile_via_ctypes
    except ImportError:
        return
    mod = types.ModuleType("antenv.axon_hooks")
    mod._hook = None

    def set_axon_ntff_profile_hook(h):
        mod._hook = h

    def get_axon_ntff_profile_hook():
        return mod._hook

    mod.set_axon_ntff_profile_hook = set_axon_ntff_profile_hook
    mod.get_axon_ntff_profile_hook = get_axon_ntff_profile_hook
    sys.modules["antenv.axon_hooks"] = mod
    antenv.axon_hooks = mod
    for so in ("/opt/axon/libaxon_pjrt.so",):
        if os.path.exists(so):
            try:
                mod._hook = _ntff_profile_via_ctypes(so)
            except Exception:
                mod._hook = None
            break


def _install_drain_patch():
    """Work around this toolchain's walrus rejecting sem waits on Drain.

    TileContext's final drain carries end-of-kernel semaphore waits inline;
    this walrus build encodes Drain as NEURON_ISA_TPB_CTRL_NO_STRUCT and
    fails codegen ("Too many sync wait commands") for ANY inline wait.
    Equivalent semantics: emit the waits as standalone sync-engine wait
    instructions and leave the Drain bare.
    """
    if getattr(tile.TileContext, "_drain_patch_installed", False):
        return
    from concourse.vector_clock import ScopedClock

    def _patched_drain_and_barrier(self, tick_clock, wait_clock):
        drain_inst = self.nc.sync.drain()
        wait_clock.add_sem_waits(
            drain_inst.ins, ScopedClock({None: tick_clock.global_clock})
        )
        si = drain_inst.ins.sync_info
        waits = list(si.on_wait) if si is not None else []
        if waits:
            drain_inst.ins.sync_info = mybir.SyncInfo(
                on_wait=[], on_update=list(si.on_update) if si.on_update else []
            )
            by_name = (
                {h.name: h for h in self.sems.allocated().values()}
                if self.sems is not None else {}
            )
            for w in waits:
                sem = by_name.get(w.ant_name)
                assert sem is not None, f"unknown drain-wait sem: {w.ant_name}"
                assert w.wait_mode == "sem-ge-imm", w
                self.nc.sync.wait_ge(sem, w.wait_value)
        self.nc.all_engine_barrier()
        assert self.sems is not None
        popped = self.nc._tile_sem_poison_stack.pop()
        assert popped is self._sem_poison
        self.nc.clear_and_free_semaphores(list(self.sems.allocated().values()))
        self.nc.all_engine_barrier()

    tile.TileContext._drain_and_barrier = _patched_drain_and_barrier

    # Same walrus limitation, general form: at most ONE inline sem wait per
    # instruction. Tile's wait assignment can attach several (e.g. a DMA
    # waiting on a slot freed by PE + DVE + another queue). Hoist all but
    # the last wait onto same-engine EventSemaphore carrier instructions.
    orig_add = tile.TileContext._add_instruction

    def _split_add_instruction(self, inst):
        si = inst.sync_info
        if si is not None and si.on_wait and len(si.on_wait) > 1:
            waits = list(si.on_wait)
            for w in waits[:-1]:
                ev = mybir.InstEventSemaphore(
                    name=self.nc.get_next_instruction_name(),
                    engine=inst.engine,
                    sync_info=mybir.SyncInfo(on_wait=[w], on_update=[]),
                )
                orig_add(self, ev)
            inst.sync_info = mybir.SyncInfo(
                on_wait=[waits[-1]],
                on_update=list(si.on_update) if si.on_update else [],
            )
        orig_add(self, inst)

    tile.TileContext._add_instruction = _split_add_instruction
    tile.TileContext._drain_patch_installed = True


def build_core_kernel(S=1024, n_pairs=4, e_out=1024):
    """Build the per-core Bass program (SPMD: same program on all cores)."""
    _install_drain_patch()
    C = S // 128            # t-chunks (t = c*128 + p)
    NT = min(512, S)        # matmul moving free-dim tile
    NS = S // NT            # s-tiles
    S4 = S // 4             # denom s-group width (DoubleRow rhs free = 2*S4/2)
    EC = e_out // 128       # output e-chunks
    FP = n_pairs * 128      # feature rows handled by this core
    f32, f16, f8 = _f32, _f16, _f8
    DR = mybir.MatmulPerfMode.DoubleRow

    nc = bass.Bass()
    q_rows = nc.declare_dram_parameter("q_rows", [FP, S], f16, isOutput=False)
    k_rows = nc.declare_dram_parameter("k_rows", [FP, S], f16, isOutput=False)
    v_rows = nc.declare_dram_parameter("v_rows", [FP, S], f16, isOutput=False)
    # weights already in on-chip layout (partition-first) for contiguous DMA,
    # packed into one tensor (2KB rows -> fewer, larger DMA descriptors).
    # Slot 0 holds N_h = Wq[h]^T @ Wk[h] (host-folded): pre = q_raw^T N k_raw,
    # so only K needs an on-chip projection and QK's rhs is the RAW q rows.
    wnv = nc.declare_dram_parameter("wnv", [128, 2, n_pairs, 128], f16,
                                    isOutput=False)
    woT = nc.declare_dram_parameter("woT", [128, n_pairs, e_out], f16, isOutput=False)
    # two partial outputs (fc-chunks 01 / 23); host sums them in fp32
    out_pa = nc.declare_dram_parameter("out_pa", [e_out, S], f16, isOutput=True)
    out_pb = nc.declare_dram_parameter("out_pb", [e_out, S], f16, isOutput=True)

    Exp = mybir.ActivationFunctionType.Exp
    Mult = mybir.AluOpType.mult
    Add = mybir.AluOpType.add

    with tile.TileContext(nc) as tc, ExitStack() as ctx:
        raws = ctx.enter_context(tc.tile_pool(name="raws", bufs=8))
        wop = ctx.enter_context(tc.tile_pool(name="wop", bufs=1))
        consts = ctx.enter_context(tc.tile_pool(name="consts", bufs=1))
        qks = ctx.enter_context(tc.tile_pool(name="qks", bufs=1))
        vts = ctx.enter_context(tc.tile_pool(name="vts", bufs=n_pairs))
        kpads = ctx.enter_context(tc.tile_pool(name="kpads", bufs=2))
        Epool = ctx.enter_context(tc.tile_pool(name="Epool", bufs=4))
        outp = ctx.enter_context(tc.tile_pool(name="outp", bufs=1))
        rcp = ctx.enter_context(tc.tile_pool(name="rcp", bufs=2))
        dstp = ctx.enter_context(tc.tile_pool(name="dstp", bufs=2))
        wostp = ctx.enter_context(tc.tile_pool(name="wostp", bufs=8))
        trp = ctx.enter_context(tc.tile_pool(name="trp", bufs=2))
        dram = ctx.enter_context(tc.tile_pool(name="dscratch", bufs=4, space="DRAM"))
        # 2 two-bank QK psum slots (one chunk: head h0's exp frees its slot
        # while h1's is still draining, so QK(c+1) can always proceed);
        # denominators get their own pool so they never gate the exp stream.
        pqk = ctx.enter_context(tc.tile_pool(name="pqk", bufs=2, space="PSUM"))
        psm = ctx.enter_context(tc.tile_pool(name="psm", bufs=2, space="PSUM"))
        dpsp = ctx.enter_context(tc.tile_pool(name="dpsp", bufs=2, space="PSUM"))

        ones8 = consts.tile([128, 2, 32], f8, tag="ones8")
        nc.vector.memset(ones8, 1.0)
        # warm-up activation: pulls the ~1.3us exp ACT_TABLE_LOAD into the
        # DMA-bound startup instead of stalling the first real exp
        warm = consts.tile([1, 8], f32, tag="warm")
        nc.vector.memset(warm, 0.0)
        nc.scalar.activation(out=warm, in_=warm, func=Exp)
        # PE warm-up: ~4us of junk matmuls during the DMA-bound startup so
        # the HAM clock gate reaches 8/8 before the first real matmul
        wjunk = consts.tile([128, NT], f16, tag="wjunk")
        nc.gpsimd.memset(wjunk, 0.0)
        wps = pqk.tile([128, NS * NT], f32, tag="pqk", name="warmps")
        for i in range(24):
            nc.tensor.matmul(wps[:, 0:NT], lhsT=wjunk[:, 0:128], rhs=wjunk,
                             start=True, stop=True)

        w_sb = consts.tile([128, 2, n_pairs, 128], f16, tag="wnv")
        wk_sb = w_sb[:, 0]
        wv_sb = w_sb[:, 1]

        k_all = qks.tile([128, n_pairs, S], f16, tag="qall")
        out_all = outp.tile([128, n_pairs, S], f16, tag="outall")

        # ---- input loads.  The critical first-pair tensors go out in
        # parallel on the sync and scalar queues (each dma_start costs
        # ~0.6us of its engine's time); the rest split sync/gpsimd. ----
        raw_tiles = []
        for pr in range(n_pairs):
            qr = raws.tile([128, S], f16, tag="raw", name=f"qr{pr}")
            kr = raws.tile([128, S], f16, tag="raw", name=f"kr{pr}")
            vr = raws.tile([128, S], f16, tag="raw", name=f"vr{pr}")
            raw_tiles.append((qr, kr, vr))

        kpad_tiles = {}
        for pr in (0, 1):
            kp = kpads.tile([128, 2, C, 128], f16, tag="kpad",
                            name=f"kpad{pr}")
            kpad_tiles[pr] = kp
            nc.vector.memset(kp, 0.0)

        qr0, kr0, vr0 = raw_tiles[0]
        nc.sync.dma_start(out=w_sb, in_=wnv[:, :, :, :])
        nc.scalar.dma_start(out=kr0[:, 0:NT], in_=k_rows[0:128, 0:NT])
        nc.gpsimd.dma_start(out=kr0[:, NT:S], in_=k_rows[0:128, NT:S])
        nc.sync.dma_start(out=qr0[:, 0:NT], in_=q_rows[0:128, 0:NT])
        nc.scalar.dma_start(out=qr0[:, NT:S], in_=q_rows[0:128, NT:S])
        # kr1 outranks vr0: the k-projections gate the windows while the
        # pv thunks are scheduled late
        nc.scalar.dma_start(out=raw_tiles[1][1], in_=k_rows[128:256, :])
        nc.scalar.dma_start(out=vr0, in_=v_rows[0:128, :])
        for pr in range(1, n_pairs):
            qr, kr, vr = raw_tiles[pr]
            eng = (nc.scalar, nc.gpsimd, nc.sync)[pr - 1]
            if pr != 1:
                eng.dma_start(out=kr, in_=k_rows[pr * 128:(pr + 1) * 128, :])
            eng.dma_start(out=qr, in_=q_rows[pr * 128:(pr + 1) * 128, :])
            eng.dma_start(out=vr, in_=v_rows[pr * 128:(pr + 1) * 128, :])
        # woT is not needed until the first Wo chunks (window 2): issue it
        # after the pair loads so they win the DMA-descriptor race
        woT_sb = wop.tile([128, n_pairs, e_out], f16, tag="woT")
        nc.gpsimd.dma_start(out=woT_sb, in_=woT[:, :, :])

        vt_tiles = {}
        scr_hist = {}

        # ---- projections (per pair, as interleavable thunks) ----
        def proj_thunks(pr):
            qr, kr, vr = raw_tiles[pr]

            def mk_qk(src, dst, wt):
                def th():
                    for st in range(NS):
                        ps = psm.tile([128, NT], f32, tag="ps")
                        nc.tensor.matmul(
                            ps, lhsT=wt[:, pr, :],
                            rhs=src[:, st * NT:(st + 1) * NT],
                            start=True, stop=True)
                        nc.vector.tensor_copy(
                            out=dst[:, pr, st * NT:(st + 1) * NT], in_=ps)
                return th

            def pk_pad():
                # Zero-padded per-head QK stationaries: K=64 matmuls run at
                # HALF rate, but a K=128 stationary whose other half is
                # zeros streams at full rate.  Zero halves are never
                # overwritten -> memset each pool slot once (pairs 0/1,
                # hoisted to kernel start).
                if pr < 2:
                    kp = kpad_tiles[pr]
                else:
                    kp = kpads.tile([128, 2, C, 128], f16, tag="kpad",
                                    name=f"kpad{pr}")
                    kpad_tiles[pr] = kp
                for hh in (0, 1):
                    sl = slice(64 * hh, 64 * hh + 64)
                    nc.vector.tensor_copy(
                        out=kp[sl, hh, :, :],
                        in_=k_all[sl, pr, :].rearrange(
                            "j (c t) -> j c t", c=C))

            def pv():
                vt = vts.tile([128, C, 128], f16, tag="vt", name=f"vt{pr}")
                vt_tiles[pr] = vt
                vrc = vr.rearrange("p (c t) -> p c t", c=C)
                for c in range(0, C, 2):
                    ps = psm.tile([128, NT], f32, tag="ps")
                    for j in (0, 1):
                        nc.tensor.matmul(
                            ps[:, 128 * j:128 * j + 128],
                            lhsT=vrc[:, c + j, :], rhs=wv_sb[:, pr, :],
                            start=True, stop=True)
                    nc.vector.tensor_copy(
                        out=vt[:, c:c + 2, :], in_=ps[:, :256])

            return [mk_qk(kr, k_all, wk_sb), pk_pad, pv]

        # ---- phase 3 pieces ----
        def emit_qk_chunk(pr, Es, c):
            kp = kpad_tiles[pr]
            qr = raw_tiles[pr][0]
            pst = [pqk.tile([128, NS * NT], f32, tag="pqk",
                            name=f"pqk_{pr}_{c}_{i}") for i in (0, 1)]
            for hh in (0, 1):
                for st in range(NS):
                    nc.tensor.matmul(
                        pst[hh][:, st * NT:(st + 1) * NT],
                        lhsT=kp[:, hh, c, :],
                        rhs=qr[:, st * NT:(st + 1) * NT],
                        start=True, stop=True)
            for hh in (0, 1):
                # E[t, s] = exp(preT[t, s] / 8)
                nc.scalar.activation(
                    out=Es[hh][:, c, :], in_=pst[hh][:],
                    func=Exp, scale=0.125)

        dps_box = {}
        dstp_box = {}

        def emit_denom(pr, Es, hh, q2):
            """Partition sums of E[hh] via DoubleRow fp8 ones-matmul over the
            e5m2 high-byte view, one 512-wide s-group per call (two calls
            per head so QK chunks can slip in between).  Each s-group gets
            its own small psm-pool tile so the denominator never steals
            the QK psum slots that gate the exp stream."""
            if q2 == 0:
                dps_box[(pr, hh)] = []
            dps = dpsp.tile([32, NT], f32, tag="dps",
                           name=f"dps_{pr}_{hh}_{q2}")
            dps_box[(pr, hh)].append(dps)
            sl = slice(q2 * NT, (q2 + 1) * NT)
            Ev = Es[hh].bitcast(f8)[:, :, 1::2]
            for cp in range(C // 2):
                nc.tensor.matmul(
                    dps[:, :],
                    lhsT=ones8,
                    rhs=Ev[:, 2 * cp:2 * cp + 2, sl],
                    start=(cp == 0), stop=(cp == C // 2 - 1),
                    perf_mode=DR,
                    skip_group_check=True)


        tree_box = {}

        def tree_a(pr, Es, hh):
            """Denominator assist for off-tail pairs: halve the PE's DR
            ones-matmul work by pre-summing chunk pairs on the DVE.  The
            e5m2-truncation bias of sums-of-2 matches single E values to
            5e-5 (both ~log-uniform mantissas), so CORR stays valid."""
            T = trp.tile([128, 4, S], f16, tag="tree", name=f"T_{pr}_{hh}")
            tree_box[(pr, hh)] = T
            for j in range(4):
                nc.vector.tensor_tensor(
                    out=T[:, j, :], in0=Es[hh][:, 2 * j, :],
                    in1=Es[hh][:, 2 * j + 1, :], op=Add)

        def tree_b(pr, hh):
            """DR fp8 ones-matmul over the tree tile's high bytes: 4
            matmuls instead of emit_denom's 8.  (A deeper DVE tree tested
            worse: the longer denominator latency stalls the AVs.)"""
            T = tree_box[(pr, hh)]
            Tv = T.bitcast(f8)[:, :, 1::2]
            dps_box[(pr, hh)] = []
            for q2 in range(NS):
                dps = dpsp.tile([32, NT], f32, tag="dps",
                               name=f"tdps_{pr}_{hh}_{q2}")
                dps_box[(pr, hh)].append(dps)
                sl = slice(q2 * NT, (q2 + 1) * NT)
                for j in (0, 1):
                    nc.tensor.matmul(
                        dps[:, :],
                        lhsT=ones8,
                        rhs=Tv[:, 2 * j:2 * j + 2, sl],
                        start=(j == 0), stop=(j == 1),
                        perf_mode=DR,
                        skip_group_check=True)

        def recip_scale(pr, hh):
            # The denominator row (all 32 dps rows are identical) becomes
            # partition-major [128, C] entirely on-chip: one 32x32-block
            # StreamTranspose + 4 strided picks.  tq[p, 32b+j] =
            # denom[32b+p], so rcr[32a+p, c] = tq[p, 128c+32a].
            dpsl = dps_box[(pr, hh)]
            tq = dstp.tile([32, S], f32, tag="dstage", name=f"tq_{pr}_{hh}")
            for q2 in range(NS):
                nc.vector.transpose(
                    out=tq[:, q2 * NT:(q2 + 1) * NT], in_=dpsl[q2][:, :])
            rcr = rcp.tile([128, C], f32, tag="rcraw", name=f"rcr_{pr}_{hh}")
            rc = rcp.tile([128, C], f32, tag="rc", name=f"rc_{pr}_{hh}")
            for a in range(4):
                nc.vector.tensor_copy(
                    out=rcr[32 * a:32 * a + 32, :],
                    in_=tq[:, 32 * a::128])
            nc.vector.reciprocal(out=rc, in_=rcr)
            vt = vt_tiles[pr]
            nc.vector.tensor_tensor(
                out=vt[:, :, 64 * hh:64 * hh + 64],
                in0=vt[:, :, 64 * hh:64 * hh + 64],
                in1=rc[:, :, None].to_broadcast((128, C, 64)),
                op=Mult)

        av_box = {}

        def av_st(pr, Es, st, hh):
            # per-head 8-deep psum chains (partition halves of one tile);
            # the h0 chain depends only on recip(h0), so it overlaps the
            # h1 denominator round-trip
            if hh == 0:
                av_box[(pr, st)] = psm.tile([128, NT], f32, tag="ps",
                                            name=f"avp_{pr}_{st}")
            avp = av_box[(pr, st)]
            for c in range(C):
                nc.tensor.matmul(
                    avp[64 * hh:64 * hh + 64, :],
                    lhsT=vt_tiles[pr][:, c, 64 * hh:64 * hh + 64],
                    rhs=Es[hh][:, c, st * NT:(st + 1) * NT],
                    start=(c == 0), stop=(c == C - 1),
                    tile_position=(0, 64 * hh),
                    skip_group_check=True)
            if hh == 1:
                nc.vector.tensor_copy(
                    out=out_all[:, pr, st * NT:(st + 1) * NT], in_=avp)

        # Wo as two independent half-projections (fc 0+1 -> out_pa,
        # fc 2+3 -> out_pb); the host adds the partials.  Each chunk is a
        # short psum chain + fp16 cast + store, so the first half spreads
        # into the exp-bound windows and only the second half tails.
        def wo_chunk(half, ec, st, tail=False):
            fc0 = 2 * half
            dst = out_pa if half == 0 else out_pb
            ops = psm.tile([128, NT], f32, tag="ps",
                           name=f"wo{half}_{ec}_{st}")
            for i in range(2):
                nc.tensor.matmul(
                    ops,
                    lhsT=woT_sb[:, fc0 + i, ec * 128:(ec + 1) * 128],
                    rhs=out_all[:, fc0 + i, st * NT:(st + 1) * NT],
                    start=(i == 0), stop=(i == 1))
            wost = wostp.tile([128, NT], f16, tag="wost")
            if tail and ec % 2 == 0:
                # exp is done by now; scalar and vector engines split the
                # tail casts so the final drain isn't serialized on one
                nc.scalar.copy(out=wost, in_=ops)
            else:
                nc.vector.tensor_copy(out=wost, in_=ops)
            oeng = nc.gpsimd if (tail and ec % 2) else nc.sync
            oeng.dma_start(
                out=dst[ec * 128:(ec + 1) * 128, st * NT:(st + 1) * NT],
                in_=wost)

        def wo_thunks(half, ec_range):
            return [
                (lambda e=ec, s=st: wo_chunk(half, e, s))
                for ec in ec_range for st in range(NS)
            ]

        # ---- schedule ----
        # window pr hosts: QK(pr) + [denom(pr-1), recip(pr-1), av(pr-1),
        # leftover proj / first-half Wo] so the PE stays dense while ACT
        # drains the exps.
        p0 = proj_thunks(0)
        for th in p0[:2]:
            th()
        # DMA-gated thunks go LAST: a pv (waiting on vr DMA) early in the
        # in-order PE queue blocks already-runnable QK chunks behind it
        p1 = proj_thunks(1)
        p2 = proj_thunks(2) if n_pairs > 2 else []
        pending = p1[:2] + p2[:2] + [p0[2]] + p1[2:] + p2[2:]
        wo01 = wo_thunks(0, range(EC))
        Es_hist = {}
        for pr in range(n_pairs):
            E0 = Epool.tile([128, C, S], f16, tag="E", name=f"E0_{pr}")
            E1 = Epool.tile([128, C, S], f16, tag="E", name=f"E1_{pr}")
            Es = (E0, E1)
            Es_hist[pr] = Es
            n = len(pending)
            for c in range(C):
                emit_qk_chunk(pr, Es, c)
                for th in pending[(n * c) // C:(n * (c + 1)) // C]:
                    th()
            prev = pr
            if pr < n_pairs - 1:
                # non-tail pairs: DVE pre-sums start at window open; PE
                # filler (proj3 in w1, ready Wo chunks in w3) sits between
                # tree_a and tree_b so the in-order PE queue never reaches
                # tree_b before the DVE finishes the adds.  Wo chunks that
                # need THIS window's AVs must stay after them (w2).
                pad = []
                post_fill = []
                if pr == 0 and n_pairs > 3:
                    pad = proj_thunks(3)
                elif pr == 1:
                    post_fill = wo01[:4]
                elif pr == 2:
                    pad = wo01[4:6]
                    post_fill = wo01[6:10]
                pending = [
                    (lambda p=prev: tree_a(p, Es_hist[p], 0)),
                    (lambda p=prev: tree_a(p, Es_hist[p], 1)),
                ] + pad + [
                    (lambda p=prev: tree_b(p, 0)),
                    (lambda p=prev: recip_scale(p, 0)),
                    (lambda p=prev: av_st(p, Es_hist[p], 0, 0)),
                    (lambda p=prev: tree_b(p, 1)),
                    (lambda p=prev: recip_scale(p, 1)),
                    (lambda p=prev: av_st(p, Es_hist[p], 0, 1)),
                    (lambda p=prev: av_st(p, Es_hist[p], 1, 0)),
                    (lambda p=prev: av_st(p, Es_hist[p], 1, 1)),
                ] + post_fill
        # ---- tail: denom3 (low-latency DR form) + recip3, then av3/woB
        # interleaved by s-half so output stores start as early as
        # possible; leftover fc01 chunks pad the round-trip latency ----
        last = n_pairs - 1
        fill = wo01[10:]
        for th in fill[0:1]:
            th()
        emit_denom(last, Es_hist[last], 0, 0)
        emit_denom(last, Es_hist[last], 0, 1)
        # recip(h0) immediately: its DVE chain (transpose/picks/recip/
        # vt-mult) overlaps the h1 denominator matmuls below, so av(h0)
        # is ready the moment the PE gets there
        recip_scale(last, 0)
        for th in fill[1:2]:
            th()
        emit_denom(last, Es_hist[last], 1, 0)
        for th in fill[2:6]:
            th()
        emit_denom(last, Es_hist[last], 1, 1)
        # recip(h1) right away too: its DVE chain overlaps the av(h0)
        # chain, so the h1 AVs (which gate woB) never stall.  All fills
        # sit BEFORE the AV block: a fill between AVs holds a psm slot
        # (each AV st-tile lives until its h1 copy) and stalls them.
        recip_scale(last, 1)
        av_st(last, Es_hist[last], 0, 0)
        av_st(last, Es_hist[last], 1, 0)
        av_st(last, Es_hist[last], 0, 1)
        for ec in range(EC):
            wo_chunk(1, ec, 0, tail=True)
        av_st(last, Es_hist[last], 1, 1)
        for ec in range(EC):
            wo_chunk(1, ec, 1, tail=True)

    return nc


def make_in_maps(queries, keys, values, Wq, Wk, Wv, Wo):
    """Shard the full inputs into the 8 per-core input dicts."""
    queries = np.ascontiguousarray(queries, dtype=np.float32)
    keys = np.ascontiguousarray(keys, dtype=np.float32)
    values = np.ascontiguousarray(values, dtype=np.float32)
    Wq = np.asarray(Wq, dtype=np.float32)
    Wk = np.asarray(Wk, dtype=np.float32)
    Wv = np.asarray(Wv, dtype=np.float32) * CORR
    Wo = np.asarray(Wo, dtype=np.float32)
    WoT = np.ascontiguousarray(Wo.T)
    # fold Wq into the K projection: pre = q'^T k' = q_raw^T (Wq^T Wk) k_raw
    Wn = np.einsum('hij,hik->hjk', Wq, Wk)

    def blockdiag(W, head_base):
        # already in the on-chip [128, n_pairs, 128] partition-first layout
        blk = np.zeros((N_PAIRS_FULL, 128, 128), dtype=np.float32)
        for pr in range(N_PAIRS_FULL):
            h0 = head_base + 2 * pr
            blk[pr, :64, :64] = W[h0].T
            blk[pr, 64:, 64:] = W[h0 + 1].T
        return blk.transpose(1, 0, 2)

    in_maps = []
    for c in range(N_CORES):
        b, hg = c // 2, c % 2
        r0, r1 = hg * 512, (hg + 1) * 512
        head_base = hg * HEADS_PER_CORE
        wnv = np.ascontiguousarray(np.stack(
            [blockdiag(Wn, head_base), blockdiag(Wv, head_base)], axis=1))
        m = {
            "q_rows": np.ascontiguousarray(queries[b, r0:r1, :]),
            "k_rows": np.ascontiguousarray(keys[b, r0:r1, :]),
            "v_rows": np.ascontiguousarray(values[b, r0:r1, :]),
            "wnv": wnv,
            "woT": np.ascontiguousarray(
                WoT[r0:r1, :].reshape(N_PAIRS_FULL, 128, E).transpose(1, 0, 2)),
        }
        m = {k: v.astype(np.float16) for k, v in m.items()}
        in_maps.append(m)
    return in_maps


LAST_RESULT = None


def kernel(queries, keys, values, Wq, Wk, Wv, Wo):
    """Full-input entry point: shard -> run on 8 NeuronCores -> unshard."""
    global LAST_RESULT
    from concourse.bass_utils import run_bass_kernel_spmd

    trace = bool(int(os.environ.get("BASS_KERNEL_TRACE", "0")))
    if trace:
        _install_ntff_shim()

    nc = build_core_kernel(S=S_FULL, n_pairs=N_PAIRS_FULL, e_out=E)
    in_maps = make_in_maps(queries, keys, values, Wq, Wk, Wv, Wo)
    res = run_bass_kernel_spmd(nc, in_maps, core_ids=list(range(N_CORES)),
                               trace=trace)
    LAST_RESULT = res
    out = np.empty((B, E, S_FULL), dtype=np.float32)
    for b in range(B):
        acc = None
        for c in (2 * b, 2 * b + 1):
            for k in ("out_pa", "out_pb"):
                p = res.results[c][k].astype(np.float32)
                acc = p if acc is None else acc + p
        out[b] = acc
    return out

